# revision 19
# baseline (speedup 1.0000x reference)
"""Trainium2 Bass kernel for cross-attention with per-head structured mask.

Reference computation (B=4, N=1024, DIM=1024, H=16, D=64):
    q = x1 @ Wq;  k, v = split(x2 @ Wkv)
    dots = q k^T * D^-0.5 + spd
    attn = softmax(dots) * (head_keep * H / n_kept)   # whole heads dropped
    out  = (attn @ v) @ Wo + bo

Sharding: dropped heads contribute exactly zero, so only kept heads are
computed. Work unit = (batch b, kept-head group g): 8 cores = 4 batches x 2
head groups. Each core computes a partial out[b] (its heads' contribution
through Wo); host sums the two partials per batch and adds the bias.

v2 design (from perfetto evidence on v1: scalar/ACT queue was the
critical path -- 56x 512-wide exps at 829ns plus 40us of semaphore
waits; phases ran serially; DMA only ~30% utilized):
  - exp and the espd multiply are batched 1024-wide: score matmuls for
    m-tile pairs land in one 2-bank PSUM tile ([128,2,512] f32), one
    ACTIVATE covers both (1114ns vs 2x687), one tensor_tensor multiplies
    by exp(spd) (exp(s+p) = exp(s)*exp(p), bias as elementwise mult).
  - the multiply is split between DVE and GpSimd (gpsimd tensor_tensor
    on all-SBUF bf16 measured ~1.3us/1024-wide; gpsimd is otherwise
    idle, so this is free parallel capacity).
  - phase A (projections) is software-pipelined INTO phase B: only
    kv(m-half 0) + q(n-half 0) precede the first attention unit; the
    remaining projection matmul groups are interleaved between B's
    score/ctx matmuls in the PE queue so the ACT exp stream never
    starves and the PE never idles (HAM stays warm).
  - one PSUM pool of [128,2,512] tiles (bufs=3, 6 banks) is shared by
    projection accumulation chains, score pairs, and out-proj pairs;
    ctx gets its own 2 banks. PSUM->SBUF evacuation copies run on the
    scalar engine during its idle ramp window and on DVE mid-stream.
  - V-augmentation parity is fixed (ones columns always 0:63, values
    64:127) so the softmax denominator always lands at partition base 0:
    reciprocal_approx_fast (base-0-only custom op) reads ctx PSUM
    directly, no shuttle copy; V copy-out and the ones memset collapse
    to single strided ops.
  - startup DMA is issued k-slab-granular (128KB descriptors) round-
    robin over sync/scalar/gpsimd queues in dependency-priority order
    (wk, xk half 0, wq, xq half 0, wv, spd unit 0, ...), so ~15+
    descriptors are in flight immediately (per-descriptor throughput is
    only 12-25GB/s; aggregate needs concurrency).

HW quirks baked in (empirical, v1+v2 sessions):
  - custom-DVE ops (reciprocal_approx_fast) only behave at partition
    base 0.
  - fp32/fp32r matmuls run at ~2-4 cyc/row; bf16 at ~1; operands bf16.
  - fp8 espd fails the 2e-2 absmax gate (2.16e-2 even half-fp8).
  - engines execute queues IN PROGRAM ORDER; emission order is the
    schedule. Outstanding DMA descriptors share bandwidth concurrently.
  - only sync (SP), scalar (Activation) and gpsimd can issue DMA
    (~650ns per descriptor on the issuing queue).
  - ACTIVATE cost ~= (N+352)/1.2 ns; batching amortizes the overhead.
  - multi-bank PSUM tiles are legal and ACT/DVE read across banks fine
    (matmul writes stay within one bank).
  - dense back-to-back PE streams can power-throttle; device drifts
    ~20% slower when hot from repeated benching.
"""

import os

import numpy as np

B, N, DIM = 4, 1024, 1024
HEADS, DIM_HEAD = 16, 64
INNER = HEADS * DIM_HEAD
SCALE = DIM_HEAD ** -0.5
NCORES = 8
KT = DIM // 128      # 8 contraction tiles
NB = N // 512        # 2 column blocks
MT = N // 128        # 8 key tiles

_cache: dict = {}


def _build(H_c: int, keep_scale: float, mode: str = "bf16", half_last: bool = False):
    """Build + compile the per-core Bass program for H_c heads (H_c even)."""
    import concourse.mybir as mybir
    import concourse.tile as tile
    from concourse import bacc

    dt = mybir.dt
    f32 = dt.float32
    HB = H_c // 2
    HD = H_c * DIM_HEAD
    # the v3 pair-unit schedule hardcodes slot positions for HB == 2
    assert H_c == 4, f"v3 schedule requires H_c == 4, got {H_c}"

    mmdt = {"bf16": dt.bfloat16, "f32r": dt.float32r, "f32": f32}[mode]

    nc = bacc.Bacc("TRN2", target_bir_lowering=False)

    xq = nc.dram_tensor("xq", [128, KT, N], mmdt, kind="ExternalInput")   # x1[b].T, sbuf image
    xk = nc.dram_tensor("xk", [128, KT, N], mmdt, kind="ExternalInput")   # x2[b].T, sbuf image
    wq = nc.dram_tensor("wq", [128, KT, HD], mmdt, kind="ExternalInput")  # sbuf layout
    wk = nc.dram_tensor("wk", [128, KT, HD], mmdt, kind="ExternalInput")
    wv = nc.dram_tensor("wv", [128, KT, HD], mmdt, kind="ExternalInput")
    wo = nc.dram_tensor("wo", [128, HD // 128, DIM], mmdt, kind="ExternalInput")
    # exp(spd[b,h]).T images, interleaved per head-PAIR: index [hb, n-block,
    # partition, m-tile, parity, n] so one [128, 2, 512] slice covers both
    # heads of a pair for one m-tile (feeds one 1024-wide exp/mult batch)
    spddt = mmdt
    spd = nc.dram_tensor("spd", [HB, NB, 128, MT, 2, 512], spddt,
                         kind="ExternalInput")
    # bf16 partials: host sums the two per-batch partials in fp32
    out = nc.dram_tensor("out", [N, DIM], dt.bfloat16, kind="ExternalOutput")

    Exp = mybir.ActivationFunctionType.Exp
    mult = mybir.AluOpType.mult

    # DMA issue queues, round-robined for startup concurrency
    with tile.TileContext(nc) as tc:
        with (
            tc.tile_pool(name="w", bufs=1) as wpool,
            tc.tile_pool(name="big", bufs=1) as big,
            tc.tile_pool(name="spdp", bufs=4) as spdp,
            tc.tile_pool(name="es", bufs=3) as espool,
            tc.tile_pool(name="ep", bufs=4) as epool,
            tc.tile_pool(name="nrm", bufs=2) as nrm,
            tc.tile_pool(name="op", bufs=3) as opool,
            tc.tile_pool(name="psS", bufs=2, space="PSUM") as psS,
            tc.tile_pool(name="psC", bufs=2, space="PSUM") as psC,
        ):
            wq_sb = wpool.tile([128, KT, HD], mmdt, tag="wq")
            wk_sb = wpool.tile([128, KT, HD], mmdt, tag="wk")
            wv_sb = wpool.tile([128, KT, HD], mmdt, tag="wv")
            wo_sb = wpool.tile([128, HD // 128, DIM], mmdt, tag="wo")
            xk_sb = big.tile([128, KT, N], mmdt, tag="xkim")
            xq_sb = big.tile([128, KT, N], mmdt, tag="xqim")

            qt_sb = big.tile([128, HB, N], mmdt, tag="qt")
            kt_sb = big.tile([128, HB, N], mmdt, tag="kt")
            v_sb = big.tile([128, MT, H_c * 128], mmdt, tag="v")
            ct_sb = big.tile([128, HB, N], mmdt, tag="ct")
            vh = v_sb.rearrange("p m (h x) -> p m h x", h=H_c)

            # ---- startup DMA: dependency-priority order, 128KB slices,
            # round-robin over the three issue-capable queues ----
            qs = [nc.sync, nc.gpsimd]
            qi = [0]

            def issue(dst, src):
                qs[qi[0] % 2].dma_start(dst, src)
                qi[0] += 1

            issue(wk_sb[:, 0:KT // 2, :], wk[:, 0:KT // 2, :])
            issue(wk_sb[:, KT // 2:, :], wk[:, KT // 2:, :])
            for k in range(KT):
                issue(xk_sb[:, k, 0:512], xk[:, k, 0:512])
            issue(wq_sb[:, 0:KT // 2, :], wq[:, 0:KT // 2, :])
            issue(wq_sb[:, KT // 2:, :], wq[:, KT // 2:, :])
            issue(wv_sb[:, 0:KT // 2, :], wv[:, 0:KT // 2, :])
            issue(wv_sb[:, KT // 2:, :], wv[:, KT // 2:, :])
            for k in range(KT):
                issue(xq_sb[:, k, 0:512], xq[:, k, 0:512])
            # spd pair-unit tiles: [128, MT, 2, 512] (full pair) or
            # [128, MT, 512] strided fetch of parity 0 only (half unit)
            spd_tiles: dict = {}

            def spd_issue(hb, n0, half_unit):
                if half_unit:
                    t = spdp.tile([128, MT, 512], spddt, tag="spd",
                                  name=f"spd{hb}_{n0}h")
                    for qq in range(4):
                        nc.sync.dma_start(
                            t[:, 2 * qq:2 * qq + 2, :],
                            spd[hb, n0, :, 2 * qq:2 * qq + 2, 0, :])
                else:
                    t = spdp.tile([128, MT, 2, 512], spddt, tag="spd",
                                  name=f"spd{hb}_{n0}")
                    for qq in range(4):
                        nc.sync.dma_start(
                            t[:, 2 * qq:2 * qq + 2, :, :],
                            spd[hb, n0, :, 2 * qq:2 * qq + 2, :, :])
                spd_tiles[(hb, n0)] = t
                return t

            spd_issue(0, 0, False)
            for k in range(KT):
                issue(xk_sb[:, k, 512:N], xk[:, k, 512:N])
            if HB > 1:
                spd_issue(1, 0, False)
            issue(wo_sb[:, :, 0:512], wo[:, :, 0:512])
            issue(wo_sb[:, :, 512:1024], wo[:, :, 512:1024])
            for k in range(KT):
                issue(xq_sb[:, k, 512:N], xq[:, k, 512:N])

            # ones columns of the augmented V blocks: always cols 0:63 so
            # the sumexp rows always land at PSUM partition base 0
            nc.gpsimd.memset(vh[:, :, :, 0:64], 1.0)
            if half_last:
                # the shared head's slot is only computed at local n0=0; its
                # n0=1 region of ctxT must read as zero in the out projection
                nc.gpsimd.memset(ct_sb[64:128, HB - 1, 512:N], 0.0)

            # ---- phase A building blocks (each: matmul group + evacuation) ----
            def kq_pair_mm(ps, w_sb, x_im, half):
                """16 MMs: project K or Q for both hb blocks of one 512-col
                half.  ps: [128,2,512] psum tile."""
                sl = slice(half * 512, (half + 1) * 512)
                for k in range(KT):
                    for hb in range(HB):
                        nc.tensor.matmul(
                            ps[:, hb, :],
                            w_sb[:, k, hb * 128:(hb + 1) * 128],
                            x_im[:, k, sl],
                            start=(k == 0), stop=(k == KT - 1),
                        )

            def kq_copy(eng, ps, dst_sb, half):
                sl = slice(half * 512, (half + 1) * 512)
                if eng is nc.vector:
                    eng.tensor_copy(dst_sb[:, 0:HB, sl], ps[:, 0:HB, :])
                else:
                    eng.copy(dst_sb[:, 0:HB, sl], ps[:, 0:HB, :])

            def v_quad_mm(ps4, half, lo):
                """16 MMs: V projection for m-tiles half*4+lo .. +lo+1.
                ps4: [128,4,256] view of a [128,2,512] psum tile.  The two
                chains share one PSUM bank and start=True clears the WHOLE
                bank's has_written bits, so only the very first MM into the
                bank may carry start; the second chain's k=0 write lands via
                has_written=0 overwrite semantics."""
                sl = slice(half * 512, (half + 1) * 512)
                for k in range(KT):
                    for mi in (lo, lo + 1):
                        nc.tensor.matmul(
                            ps4[:, mi, :],
                            xk_sb[:, k, sl][:, mi * 128:(mi + 1) * 128],
                            wv_sb[:, k, :],
                            start=(k == 0 and mi == lo),
                            stop=(k == KT - 1),
                        )

            def v_quad_copy(eng, ps4, half):
                src = ps4.rearrange("p mi (h d) -> p mi h d", h=H_c)
                dst = vh[:, half * 4:(half + 1) * 4, :, 64:128]
                if eng is nc.vector:
                    eng.tensor_copy(dst, src)
                else:
                    eng.copy(dst, src)

            # ---- phase B: head-PAIR units.  Both parities of one hb are
            # processed together per m-tile: the two K=64 score MMs sit at
            # partition bases 0 and 64, so the PE runs them CONCURRENTLY in
            # one ~259ns slot (row-tile auto-packing); one 1024-wide exp and
            # one 1024-wide espd multiply cover both heads. ----
            def pair_scores(hb, n0, m, half_unit):
                n_sl = slice(n0 * 512, (n0 + 1) * 512)
                sc = psS.tile([128, 2, 512], f32, tag="ps",
                              name=f"sc{hb}_{n0}_{m}")
                nc.tensor.matmul(
                    sc[:, 0, :],
                    kt_sb[0:64, hb, m * 128:(m + 1) * 128],
                    qt_sb[0:64, hb, n_sl],
                    start=True, stop=True,
                )
                if not half_unit:
                    nc.tensor.matmul(
                        sc[:, 1, :],
                        kt_sb[64:128, hb, m * 128:(m + 1) * 128],
                        qt_sb[64:128, hb, n_sl],
                        start=True, stop=True,
                    )
                return sc

            def pair_expmul(m, sc, spds, mul_eng, half_unit):
                if half_unit:
                    es = espool.tile([128, 2, 512], mmdt, tag="es")
                    nc.scalar.activation(es[:, 0, :], sc[:, 0, :], Exp)
                    e = epool.tile([128, 2, 512], mmdt, tag="e")
                    mul_eng.tensor_tensor(
                        e[:, 0, :], es[:, 0, :], spds[:, m, :], op=mult)
                    return e
                es = espool.tile([128, 2, 512], mmdt, tag="es")
                nc.scalar.activation(
                    es[:].rearrange("p a b -> p (a b)"),
                    sc[:].rearrange("p a b -> p (a b)"), Exp)
                e = epool.tile([128, 2, 512], mmdt, tag="e")
                mul_eng.tensor_tensor(
                    e[:].rearrange("p a b -> p (a b)"),
                    es[:].rearrange("p a b -> p (a b)"),
                    spds[:, m, :, :].rearrange("p a b -> p (a b)"),
                    op=mult)
                return e

            def pair_ctx(hb, m, e, ctxp, half_unit, start, stop):
                for par in range(1 if half_unit else 2):
                    h = 2 * hb + par
                    nc.tensor.matmul(
                        ctxp[:, par, :],
                        v_sb[:, m, h * 128:(h + 1) * 128],
                        e[:, par, :],
                        start=start,
                        stop=stop,
                    )

            def pair_norm(hb, n0, ctxp, half_unit):
                n_sl = slice(n0 * 512, (n0 + 1) * 512)
                rr = nrm.tile([128, 2, 512], f32, tag="rr")
                if half_unit:
                    nc.vector.reciprocal_approx_fast(
                        rr[0:64, 0, :], ctxp[0:64, 0, :])
                else:
                    nc.vector.reciprocal_approx_fast(
                        rr[0:64, :, :].rearrange("p a b -> p (a b)"),
                        ctxp[0:64, :, :].rearrange("p a b -> p (a b)"))
                for par in range(1 if half_unit else 2):
                    nc.vector.scalar_tensor_tensor(
                        out=ct_sb[par * 64:par * 64 + 64, hb, n_sl],
                        in0=ctxp[64:128, par, :],
                        scalar=float(keep_scale),
                        in1=rr[0:64, par, :],
                        op0=mult,
                        op1=mult,
                    )

            # ---- phase C: out projection for one row-tile ----
            def c_block(nt, cast_eng, wr_eng):
                po = psS.tile([128, 2, 512], f32, tag="ps", name=f"po{nt}")
                for d0 in range(2):
                    for kk in range(HD // 128):
                        nc.tensor.matmul(
                            po[:, d0, :],
                            ct_sb[:, kk, nt * 128:(nt + 1) * 128],
                            wo_sb[:, kk, d0 * 512:(d0 + 1) * 512],
                            start=(kk == 0),
                            stop=(kk == HD // 128 - 1),
                        )
                ot = opool.tile([128, 2, 512], dt.bfloat16, tag="o")
                if cast_eng is nc.vector:
                    cast_eng.tensor_copy(
                        ot[:].rearrange("p a b -> p (a b)"),
                        po[:].rearrange("p a b -> p (a b)"))
                else:
                    cast_eng.copy(
                        ot[:].rearrange("p a b -> p (a b)"),
                        po[:].rearrange("p a b -> p (a b)"))
                for d0 in range(2):
                    if len(wr_eng) > 1:
                        for rh in range(2):
                            wr_eng[(2 * d0 + rh) % len(wr_eng)].dma_start(
                                out[nt * 128 + rh * 64:nt * 128 + rh * 64 + 64,
                                    d0 * 512:(d0 + 1) * 512],
                                ot[rh * 64:rh * 64 + 64, d0, :])
                    else:
                        wr_eng[0].dma_start(
                            out[nt * 128:(nt + 1) * 128,
                                d0 * 512:(d0 + 1) * 512],
                            ot[:, d0, :])

            # =========== EMISSION SCHEDULE ===========
            # Phase A prefix: ONLY K half 0 + Q half 0 precede phase B (the
            # V projection and all remaining A groups drop into slot
            # positions inside the B units, keeping the PE queue dense).
            # PE is the binding engine (~259ns/N=512 slot, K=64 score pairs
            # share a slot), so ACT/DVE idling mid-stream is acceptable;
            # the PE queue must never wait.
            kps0 = psS.tile([128, 2, 512], f32, tag="ps", name="kps0")
            kq_pair_mm(kps0, wk_sb, xk_sb, 0)
            kq_copy(nc.scalar, kps0, kt_sb, 0)

            qps0 = psS.tile([128, 2, 512], f32, tag="ps", name="qps0")
            kq_pair_mm(qps0, wq_sb, xq_sb, 0)
            kq_copy(nc.scalar, qps0, qt_sb, 0)

            # Remaining A groups as slot emitters (dependency notes: scores
            # for m>=4 need kt half 1 -> kps1 EMITTED before them; ctx m
            # needs v m-tile m -> vq blocks before those ctx MMs).
            vq_tiles = [None, None]

            def emit_vq(half, lo):
                def fn():
                    if lo == 0:
                        t = psS.tile([128, 2, 512], f32, tag="ps",
                                     name=f"vq{half}")
                        vq_tiles[half] = t.rearrange(
                            "p a (b x) -> p (a b) x", b=2)
                    v4 = vq_tiles[half]
                    v_quad_mm(v4, half, lo)
                    src = v4[:, lo:lo + 2, :].rearrange(
                        "p mi (h d) -> p mi h d", h=H_c)
                    nc.scalar.copy(
                        vh[:, half * 4 + lo:half * 4 + lo + 2, :, 64:128], src)
                return fn

            def emit_kps1():
                kps1 = psS.tile([128, 2, 512], f32, tag="ps", name="kps1")
                kq_pair_mm(kps1, wk_sb, xk_sb, 1)
                kq_copy(nc.scalar, kps1, kt_sb, 1)

            def emit_qps1():
                qps1 = psS.tile([128, 2, 512], f32, tag="ps", name="qps1")
                kq_pair_mm(qps1, wq_sb, xq_sb, 1)
                kq_copy(nc.scalar, qps1, qt_sb, 1)

            # SUPERUNIT schedule: both head-pair units of one n-block
            # advance together, m-step interleaved.  Doubles the independent
            # work between every cross-engine dependency edge (sc->exp->
            # mult->ctx), so PE never drains the frontier; A/C remnant
            # groups are spread one per superstep as queue filler.
            # half_last: unit B of n-block 1 is the half unit (parity 0).
            halfB = {0: False, 1: half_last}

            # remnant slot emitters: slot[(su, step)] -> list
            slots: dict = {
                (0, 0): [emit_vq(0, 0)],
                (0, 1): [emit_kps1],
                (0, 2): [emit_vq(0, 2)],
                (0, 3): [emit_qps1],
                (0, 4): [emit_vq(1, 0)],
                (0, 6): [emit_vq(1, 2)],
            }
            cq = list(range(4))
            for step in (1, 3, 5, 7):
                slots.setdefault((1, step), []).append(
                    lambda nt=cq.pop(0): c_block(nt, nc.scalar, [nc.sync]))

            spd_issue(0, 1, False)

            for su in range(NB):
                hu_b = halfB[su]
                spdsA = spd_tiles.pop((0, su))
                spdsB = spd_tiles.pop((1, su))
                if su + 1 < NB:
                    spd_issue(1, su + 1, halfB[su + 1])
                ctxA = psC.tile([128, 2, 512], f32, tag="ctx", name=f"cA{su}")
                ctxB = psC.tile([128, 2, 512], f32, tag="ctx", name=f"cB{su}")
                scsA = {0: pair_scores(0, su, 0, False),
                        1: pair_scores(0, su, 1, False)}
                scsB = {0: pair_scores(1, su, 0, hu_b),
                        1: pair_scores(1, su, 1, hu_b)}
                for m in range(MT):
                    eA = pair_expmul(m, scsA.pop(m), spdsA, nc.vector, False)
                    eB = pair_expmul(m, scsB.pop(m), spdsB, nc.vector, hu_b)
                    for fn in slots.get((su, m), ()):
                        fn()
                    if m + 2 < MT:
                        scsA[m + 2] = pair_scores(0, su, m + 2, False)
                        scsB[m + 2] = pair_scores(1, su, m + 2, hu_b)
                    pair_ctx(0, m, eA, ctxA, False,
                             start=(m == 0), stop=(m == MT - 1))
                    pair_ctx(1, m, eB, ctxB, hu_b,
                             start=(m == 0), stop=(m == MT - 1))
                pair_norm(0, su, ctxA, False)
                pair_norm(1, su, ctxB, hu_b)

            # tail: out projection for the remaining n-blocks
            for nt in cq:
                c_block(nt, nc.scalar, [nc.sync])
            for nt in range(4, 8):
                c_block(nt, nc.scalar, [nc.sync, nc.scalar])

    nc.finalize()
    return nc


def _get_nc(H_c: int, n_kept: int, mode: str, half_last: bool):
    key = (H_c, n_kept, mode, half_last)
    if key not in _cache:
        _cache[key] = _build(H_c, HEADS / n_kept, mode, half_last)
    return _cache[key]


def _prep_inputs(x1, x2, spd, head_keep, Wq, Wkv, Wo, mode="bf16"):
    """Slice/transpose/pad host-side into per-core input maps."""
    import ml_dtypes

    ndt = np.float32 if mode in ("f32", "f32r") else ml_dtypes.bfloat16
    kept = [int(i) for i in np.nonzero(head_keep)[0]]
    n_kept = len(kept)
    half_last = (n_kept % 2 == 1)
    if not half_last:
        H_c = n_kept // 2
        if H_c % 2:
            H_c += 1
        groups = [kept[:H_c], kept[H_c:]]
    else:
        # odd count: both cores of a pair share the last kept head, each
        # computing one n-half of it (local column order differs per core)
        K = (n_kept - 1) // 2
        shared = kept[-1]
        H_c = K + 1
        pad = []
        if H_c % 2:
            H_c += 1
            pad = [None]
        groups = [kept[:K] + pad + [shared], kept[K:2 * K] + pad + [shared]]

    Wk_full, Wv_full = Wkv[:, :INNER], Wkv[:, INNER:]

    in_maps = []
    for b in range(B):
        xqT = np.ascontiguousarray(
            x1[b].T.reshape(KT, 128, N).transpose(1, 0, 2)).astype(ndt)
        xkT = np.ascontiguousarray(
            x2[b].T.reshape(KT, 128, N).transpose(1, 0, 2)).astype(ndt)
        for g in range(2):
            heads = groups[g]
            swap = half_last and g == 1  # local n0=0 <-> global half 1
            xq_g = xqT
            if swap:
                xq_g = np.ascontiguousarray(
                    np.concatenate([xqT[:, :, 512:], xqT[:, :, :512]], axis=2))
            HD = H_c * DIM_HEAD
            wq_c = np.zeros((DIM, HD), np.float32)
            wk_c = np.zeros((DIM, HD), np.float32)
            wv_c = np.zeros((DIM, HD), np.float32)
            wo_c = np.zeros((HD, DIM), np.float32)
            # exp(spd) multiplies exp(qk); absent slots get 1.0 (identity).
            # Interleaved per head-pair: [hb, n-block, part, m-tile, parity, n]
            spd_c = np.ones((H_c // 2, NB, 128, MT, 2, 512), ndt)
            for i, h in enumerate(heads):
                if h is None:
                    continue
                sl = slice(i * DIM_HEAD, (i + 1) * DIM_HEAD)
                hs = slice(h * DIM_HEAD, (h + 1) * DIM_HEAD)
                wq_c[:, sl] = Wq[:, hs] * SCALE
                wk_c[:, sl] = Wk_full[:, hs]
                wv_c[:, sl] = Wv_full[:, hs]
                wo_c[sl, :] = Wo[hs, :]
                # exp(spd[b,h]).T -> [n-block, partition, m-tile, n] image,
                # n-blocks in the core's LOCAL column order
                im = (np.exp(spd[b, h].T).reshape(MT, 128, NB, 512)
                      .transpose(2, 1, 0, 3)).astype(ndt)
                spd_c[i // 2, :, :, :, i % 2, :] = im[::-1] if swap else im
            in_maps.append({
                "xq": xq_g,
                "xk": xkT,
                "wq": np.ascontiguousarray(
                    wq_c.reshape(KT, 128, HD).transpose(1, 0, 2)).astype(ndt),
                "wk": np.ascontiguousarray(
                    wk_c.reshape(KT, 128, HD).transpose(1, 0, 2)).astype(ndt),
                "wv": np.ascontiguousarray(
                    wv_c.reshape(KT, 128, HD).transpose(1, 0, 2)).astype(ndt),
                "wo": np.ascontiguousarray(
                    wo_c.reshape(HD // 128, 128, DIM).transpose(1, 0, 2)).astype(ndt),
                "spd": spd_c,
            })
    return in_maps, n_kept, H_c


def _run(nc, in_maps, trace=False, tmpdir=None):
    from concourse.bass_utils import run_bass_kernel_spmd

    return run_bass_kernel_spmd(
        nc, in_maps, core_ids=list(range(NCORES)), trace=trace, tmpdir=tmpdir
    )


def kernel(x1, x2, spd, head_keep, Wq, Wkv, Wo, bo, _trace=False, _tmpdir=None):
    x1 = np.asarray(x1, np.float32)
    x2 = np.asarray(x2, np.float32)
    spd = np.asarray(spd, np.float32)
    head_keep = np.asarray(head_keep)
    n_kept = int(head_keep.astype(np.int64).sum())
    if n_kept == 0:
        # reference: 16/0 = inf, 0*inf = nan everywhere
        return np.full((B, N, DIM), np.nan, np.float32)

    mode = os.environ.get("KERNEL_DTYPE", "bf16")
    in_maps, n_kept, H_c = _prep_inputs(
        x1, x2, spd, head_keep, Wq, Wkv, Wo, mode)
    half_last = (n_kept % 2 == 1)
    nc = _get_nc(H_c, n_kept, mode, half_last)
    res = _run(nc, in_maps, trace=_trace, tmpdir=_tmpdir)

    out = np.empty((B, N, DIM), np.float32)
    bo32 = np.asarray(bo, np.float32)
    for b in range(B):
        o0 = res.results[2 * b]["out"].astype(np.float32)
        o1 = res.results[2 * b + 1]["out"].astype(np.float32)
        if half_last:
            o1 = np.concatenate([o1[512:], o1[:512]], axis=0)
        out[b] = o0 + o1 + bo32
    kernel._last_results = res
    return out


# revision 20
# speedup vs baseline: 1.0681x; 1.0681x over previous
"""Trainium2 Bass kernel for cross-attention with per-head structured mask.

Reference computation (B=4, N=1024, DIM=1024, H=16, D=64):
    q = x1 @ Wq;  k, v = split(x2 @ Wkv)
    dots = q k^T * D^-0.5 + spd
    attn = softmax(dots) * (head_keep * H / n_kept)   # whole heads dropped
    out  = (attn @ v) @ Wo + bo

Sharding: dropped heads contribute exactly zero, so only kept heads are
computed. Work unit = (batch b, kept-head group g): 8 cores = 4 batches x 2
head groups. Each core computes a partial out[b] (its heads' contribution
through Wo); host sums the two partials per batch and adds the bias.

v2 design (from perfetto evidence on v1: scalar/ACT queue was the
critical path -- 56x 512-wide exps at 829ns plus 40us of semaphore
waits; phases ran serially; DMA only ~30% utilized):
  - exp and the espd multiply are batched 1024-wide: score matmuls for
    m-tile pairs land in one 2-bank PSUM tile ([128,2,512] f32), one
    ACTIVATE covers both (1114ns vs 2x687), one tensor_tensor multiplies
    by exp(spd) (exp(s+p) = exp(s)*exp(p), bias as elementwise mult).
  - the multiply is split between DVE and GpSimd (gpsimd tensor_tensor
    on all-SBUF bf16 measured ~1.3us/1024-wide; gpsimd is otherwise
    idle, so this is free parallel capacity).
  - phase A (projections) is software-pipelined INTO phase B: only
    kv(m-half 0) + q(n-half 0) precede the first attention unit; the
    remaining projection matmul groups are interleaved between B's
    score/ctx matmuls in the PE queue so the ACT exp stream never
    starves and the PE never idles (HAM stays warm).
  - one PSUM pool of [128,2,512] tiles (bufs=3, 6 banks) is shared by
    projection accumulation chains, score pairs, and out-proj pairs;
    ctx gets its own 2 banks. PSUM->SBUF evacuation copies run on the
    scalar engine during its idle ramp window and on DVE mid-stream.
  - V-augmentation parity is fixed (ones columns always 0:63, values
    64:127) so the softmax denominator always lands at partition base 0:
    reciprocal_approx_fast (base-0-only custom op) reads ctx PSUM
    directly, no shuttle copy; V copy-out and the ones memset collapse
    to single strided ops.
  - startup DMA is issued k-slab-granular (128KB descriptors) round-
    robin over sync/scalar/gpsimd queues in dependency-priority order
    (wk, xk half 0, wq, xq half 0, wv, spd unit 0, ...), so ~15+
    descriptors are in flight immediately (per-descriptor throughput is
    only 12-25GB/s; aggregate needs concurrency).

HW quirks baked in (empirical, v1+v2 sessions):
  - custom-DVE ops (reciprocal_approx_fast) only behave at partition
    base 0.
  - fp32/fp32r matmuls run at ~2-4 cyc/row; bf16 at ~1; operands bf16.
  - fp8 espd fails the 2e-2 absmax gate (2.16e-2 even half-fp8).
  - engines execute queues IN PROGRAM ORDER; emission order is the
    schedule. Outstanding DMA descriptors share bandwidth concurrently.
  - only sync (SP), scalar (Activation) and gpsimd can issue DMA
    (~650ns per descriptor on the issuing queue).
  - ACTIVATE cost ~= (N+352)/1.2 ns; batching amortizes the overhead.
  - multi-bank PSUM tiles are legal and ACT/DVE read across banks fine
    (matmul writes stay within one bank).
  - dense back-to-back PE streams can power-throttle; device drifts
    ~20% slower when hot from repeated benching.
"""

import os

import numpy as np

B, N, DIM = 4, 1024, 1024
HEADS, DIM_HEAD = 16, 64
INNER = HEADS * DIM_HEAD
SCALE = DIM_HEAD ** -0.5
NCORES = 8
KT = DIM // 128      # 8 contraction tiles
NB = N // 512        # 2 column blocks
MT = N // 128        # 8 key tiles

_cache: dict = {}


def _build(H_c: int, keep_scale: float, mode: str = "bf16", half_last: bool = False):
    """Build + compile the per-core Bass program for H_c heads (H_c even)."""
    import concourse.mybir as mybir
    import concourse.tile as tile
    from concourse import bacc

    dt = mybir.dt
    f32 = dt.float32
    HB = H_c // 2
    HD = H_c * DIM_HEAD
    # the v3 pair-unit schedule hardcodes slot positions for HB == 2
    assert H_c == 4, f"v3 schedule requires H_c == 4, got {H_c}"

    mmdt = {"bf16": dt.bfloat16, "f32r": dt.float32r, "f32": f32}[mode]

    nc = bacc.Bacc("TRN2", target_bir_lowering=False)

    xq = nc.dram_tensor("xq", [128, KT, N], mmdt, kind="ExternalInput")   # x1[b].T, sbuf image
    xk = nc.dram_tensor("xk", [128, KT, N], mmdt, kind="ExternalInput")   # x2[b].T, sbuf image
    wq = nc.dram_tensor("wq", [128, KT, HD], mmdt, kind="ExternalInput")  # sbuf layout
    wk = nc.dram_tensor("wk", [128, KT, HD], mmdt, kind="ExternalInput")
    wv = nc.dram_tensor("wv", [128, KT, HD], mmdt, kind="ExternalInput")
    wo = nc.dram_tensor("wo", [128, HD // 128, DIM], mmdt, kind="ExternalInput")
    # exp(spd[b,h]).T images, interleaved per head-PAIR: index [hb, n-block,
    # partition, m-tile, parity, n] so one [128, 2, 512] slice covers both
    # heads of a pair for one m-tile (feeds one 1024-wide exp/mult batch)
    spddt = mmdt
    spd = nc.dram_tensor("spd", [HB, NB, 128, MT, 2, 512], spddt,
                         kind="ExternalInput")
    # bf16 partials: host sums the two per-batch partials in fp32
    out = nc.dram_tensor("out", [N, DIM], dt.bfloat16, kind="ExternalOutput")

    Exp = mybir.ActivationFunctionType.Exp
    mult = mybir.AluOpType.mult

    # DMA issue queues, round-robined for startup concurrency
    with tile.TileContext(nc) as tc:
        with (
            tc.tile_pool(name="w", bufs=1) as wpool,
            tc.tile_pool(name="big", bufs=1) as big,
            tc.tile_pool(name="spdp", bufs=4) as spdp,
            tc.tile_pool(name="es", bufs=3) as espool,
            tc.tile_pool(name="ep", bufs=4) as epool,
            tc.tile_pool(name="nrm", bufs=2) as nrm,
            tc.tile_pool(name="op", bufs=3) as opool,
            tc.tile_pool(name="psS", bufs=3, space="PSUM") as psS,
            tc.tile_pool(name="psC", bufs=1, space="PSUM") as psC,
        ):
            wq_sb = wpool.tile([128, KT, HD], mmdt, tag="wq")
            wk_sb = wpool.tile([128, KT, HD], mmdt, tag="wk")
            wv_sb = wpool.tile([128, KT, HD], mmdt, tag="wv")
            wo_sb = wpool.tile([128, HD // 128, DIM], mmdt, tag="wo")
            xk_sb = big.tile([128, KT, N], mmdt, tag="xkim")
            xq_sb = big.tile([128, KT, N], mmdt, tag="xqim")

            qt_sb = big.tile([128, HB, N], mmdt, tag="qt")
            kt_sb = big.tile([128, HB, N], mmdt, tag="kt")
            v_sb = big.tile([128, MT, H_c * 128], mmdt, tag="v")
            ct_sb = big.tile([128, HB, N], mmdt, tag="ct")
            vh = v_sb.rearrange("p m (h x) -> p m h x", h=H_c)

            # ---- startup DMA: dependency-priority order, 128KB slices,
            # round-robin over the three issue-capable queues ----
            qs = [nc.sync, nc.gpsimd]
            qi = [0]

            def issue(dst, src):
                qs[qi[0] % 2].dma_start(dst, src)
                qi[0] += 1

            issue(wk_sb[:, 0:KT // 2, :], wk[:, 0:KT // 2, :])
            issue(wk_sb[:, KT // 2:, :], wk[:, KT // 2:, :])
            for k in range(KT):
                issue(xk_sb[:, k, 0:512], xk[:, k, 0:512])
            issue(wq_sb[:, 0:KT // 2, :], wq[:, 0:KT // 2, :])
            issue(wq_sb[:, KT // 2:, :], wq[:, KT // 2:, :])
            issue(wv_sb[:, 0:KT // 2, :], wv[:, 0:KT // 2, :])
            issue(wv_sb[:, KT // 2:, :], wv[:, KT // 2:, :])
            for k in range(KT):
                issue(xq_sb[:, k, 0:512], xq[:, k, 0:512])
            # spd pair-unit tiles: [128, MT, 2, 512] (full pair) or
            # [128, MT, 512] strided fetch of parity 0 only (half unit)
            spd_tiles: dict = {}

            def spd_issue(hb, n0, half_unit):
                if half_unit:
                    t = spdp.tile([128, MT, 512], spddt, tag="spd",
                                  name=f"spd{hb}_{n0}h")
                    for qq in range(4):
                        nc.sync.dma_start(
                            t[:, 2 * qq:2 * qq + 2, :],
                            spd[hb, n0, :, 2 * qq:2 * qq + 2, 0, :])
                else:
                    t = spdp.tile([128, MT, 2, 512], spddt, tag="spd",
                                  name=f"spd{hb}_{n0}")
                    for qq in range(4):
                        nc.sync.dma_start(
                            t[:, 2 * qq:2 * qq + 2, :, :],
                            spd[hb, n0, :, 2 * qq:2 * qq + 2, :, :])
                spd_tiles[(hb, n0)] = t
                return t

            spd_issue(0, 0, False)
            for k in range(KT):
                issue(xk_sb[:, k, 512:N], xk[:, k, 512:N])
            if HB > 1:
                spd_issue(1, 0, False)
            issue(wo_sb[:, :, 0:512], wo[:, :, 0:512])
            issue(wo_sb[:, :, 512:1024], wo[:, :, 512:1024])
            for k in range(KT):
                issue(xq_sb[:, k, 512:N], xq[:, k, 512:N])

            # ones columns of the augmented V blocks: always cols 0:63 so
            # the sumexp rows always land at PSUM partition base 0
            nc.gpsimd.memset(vh[:, :, :, 0:64], 1.0)
            if half_last:
                # the shared head's slot is only computed at local n0=0; its
                # n0=1 region of ctxT must read as zero in the out projection
                nc.gpsimd.memset(ct_sb[64:128, HB - 1, 512:N], 0.0)

            # ---- phase A building blocks (each: matmul group + evacuation) ----
            def kq_pair_mm(ps, w_sb, x_im, half):
                """16 MMs: project K or Q for both hb blocks of one 512-col
                half.  ps: [128,2,512] psum tile."""
                sl = slice(half * 512, (half + 1) * 512)
                for k in range(KT):
                    for hb in range(HB):
                        nc.tensor.matmul(
                            ps[:, hb, :],
                            w_sb[:, k, hb * 128:(hb + 1) * 128],
                            x_im[:, k, sl],
                            start=(k == 0), stop=(k == KT - 1),
                        )

            def kq_copy(eng, ps, dst_sb, half):
                sl = slice(half * 512, (half + 1) * 512)
                if eng is nc.vector:
                    eng.tensor_copy(dst_sb[:, 0:HB, sl], ps[:, 0:HB, :])
                else:
                    eng.copy(dst_sb[:, 0:HB, sl], ps[:, 0:HB, :])

            def v_quad_mm(ps4, half, lo):
                """16 MMs: V projection for m-tiles half*4+lo .. +lo+1.
                ps4: [128,4,256] view of a [128,2,512] psum tile.  The two
                chains share one PSUM bank and start=True clears the WHOLE
                bank's has_written bits, so only the very first MM into the
                bank may carry start; the second chain's k=0 write lands via
                has_written=0 overwrite semantics."""
                sl = slice(half * 512, (half + 1) * 512)
                for k in range(KT):
                    for mi in (lo, lo + 1):
                        nc.tensor.matmul(
                            ps4[:, mi, :],
                            xk_sb[:, k, sl][:, mi * 128:(mi + 1) * 128],
                            wv_sb[:, k, :],
                            start=(k == 0 and mi == lo),
                            stop=(k == KT - 1),
                        )

            def v_quad_copy(eng, ps4, half):
                src = ps4.rearrange("p mi (h d) -> p mi h d", h=H_c)
                dst = vh[:, half * 4:(half + 1) * 4, :, 64:128]
                if eng is nc.vector:
                    eng.tensor_copy(dst, src)
                else:
                    eng.copy(dst, src)

            # ---- phase B: head-PAIR units.  Both parities of one hb are
            # processed together per m-tile: the two K=64 score MMs sit at
            # partition bases 0 and 64, so the PE runs them CONCURRENTLY in
            # one ~259ns slot (row-tile auto-packing); one 1024-wide exp and
            # one 1024-wide espd multiply cover both heads. ----
            def pair_scores(hb, n0, m, half_unit):
                n_sl = slice(n0 * 512, (n0 + 1) * 512)
                sc = psS.tile([128, 2, 512], f32, tag="ps",
                              name=f"sc{hb}_{n0}_{m}")
                nc.tensor.matmul(
                    sc[:, 0, :],
                    kt_sb[0:64, hb, m * 128:(m + 1) * 128],
                    qt_sb[0:64, hb, n_sl],
                    start=True, stop=True,
                )
                if not half_unit:
                    nc.tensor.matmul(
                        sc[:, 1, :],
                        kt_sb[64:128, hb, m * 128:(m + 1) * 128],
                        qt_sb[64:128, hb, n_sl],
                        start=True, stop=True,
                    )
                return sc

            def pair_expmul(m, sc, spds, mul_eng, half_unit):
                if half_unit:
                    es = espool.tile([128, 2, 512], mmdt, tag="es")
                    nc.scalar.activation(es[:, 0, :], sc[:, 0, :], Exp)
                    e = epool.tile([128, 2, 512], mmdt, tag="e")
                    mul_eng.tensor_tensor(
                        e[:, 0, :], es[:, 0, :], spds[:, m, :], op=mult)
                    return e
                es = espool.tile([128, 2, 512], mmdt, tag="es")
                nc.scalar.activation(
                    es[:].rearrange("p a b -> p (a b)"),
                    sc[:].rearrange("p a b -> p (a b)"), Exp)
                e = epool.tile([128, 2, 512], mmdt, tag="e")
                mul_eng.tensor_tensor(
                    e[:].rearrange("p a b -> p (a b)"),
                    es[:].rearrange("p a b -> p (a b)"),
                    spds[:, m, :, :].rearrange("p a b -> p (a b)"),
                    op=mult)
                return e

            def pair_ctx(hb, m, e, ctxp, half_unit, start, stop):
                for par in range(1 if half_unit else 2):
                    h = 2 * hb + par
                    nc.tensor.matmul(
                        ctxp[:, par, :],
                        v_sb[:, m, h * 128:(h + 1) * 128],
                        e[:, par, :],
                        start=start,
                        stop=stop,
                    )

            def pair_norm(hb, n0, ctxp, half_unit):
                n_sl = slice(n0 * 512, (n0 + 1) * 512)
                rr = nrm.tile([128, 2, 512], f32, tag="rr")
                if half_unit:
                    nc.vector.reciprocal_approx_fast(
                        rr[0:64, 0, :], ctxp[0:64, 0, :])
                else:
                    nc.vector.reciprocal_approx_fast(
                        rr[0:64, :, :].rearrange("p a b -> p (a b)"),
                        ctxp[0:64, :, :].rearrange("p a b -> p (a b)"))
                for par in range(1 if half_unit else 2):
                    nc.vector.scalar_tensor_tensor(
                        out=ct_sb[par * 64:par * 64 + 64, hb, n_sl],
                        in0=ctxp[64:128, par, :],
                        scalar=float(keep_scale),
                        in1=rr[0:64, par, :],
                        op0=mult,
                        op1=mult,
                    )

            # ---- phase C: out projection for one row-tile ----
            def c_block(nt, cast_eng, wr_eng):
                po = psS.tile([128, 2, 512], f32, tag="ps", name=f"po{nt}")
                for d0 in range(2):
                    for kk in range(HD // 128):
                        nc.tensor.matmul(
                            po[:, d0, :],
                            ct_sb[:, kk, nt * 128:(nt + 1) * 128],
                            wo_sb[:, kk, d0 * 512:(d0 + 1) * 512],
                            start=(kk == 0),
                            stop=(kk == HD // 128 - 1),
                        )
                ot = opool.tile([128, 2, 512], dt.bfloat16, tag="o")
                if cast_eng is nc.vector:
                    cast_eng.tensor_copy(
                        ot[:].rearrange("p a b -> p (a b)"),
                        po[:].rearrange("p a b -> p (a b)"))
                else:
                    cast_eng.copy(
                        ot[:].rearrange("p a b -> p (a b)"),
                        po[:].rearrange("p a b -> p (a b)"))
                for d0 in range(2):
                    if len(wr_eng) > 1:
                        for rh in range(2):
                            wr_eng[(2 * d0 + rh) % len(wr_eng)].dma_start(
                                out[nt * 128 + rh * 64:nt * 128 + rh * 64 + 64,
                                    d0 * 512:(d0 + 1) * 512],
                                ot[rh * 64:rh * 64 + 64, d0, :])
                    else:
                        wr_eng[0].dma_start(
                            out[nt * 128:(nt + 1) * 128,
                                d0 * 512:(d0 + 1) * 512],
                            ot[:, d0, :])

            # =========== EMISSION SCHEDULE ===========
            # Phase A prefix: ONLY K half 0 + Q half 0 precede phase B (the
            # V projection and all remaining A groups drop into slot
            # positions inside the B units, keeping the PE queue dense).
            # PE is the binding engine (~259ns/N=512 slot, K=64 score pairs
            # share a slot), so ACT/DVE idling mid-stream is acceptable;
            # the PE queue must never wait.
            kps0 = psS.tile([128, 2, 512], f32, tag="ps", name="kps0")
            kq_pair_mm(kps0, wk_sb, xk_sb, 0)
            kq_copy(nc.scalar, kps0, kt_sb, 0)

            qps0 = psS.tile([128, 2, 512], f32, tag="ps", name="qps0")
            kq_pair_mm(qps0, wq_sb, xq_sb, 0)
            kq_copy(nc.scalar, qps0, qt_sb, 0)

            # Remaining A groups as slot emitters (dependency notes: scores
            # for m>=4 need kt half 1 -> kps1 EMITTED before them; ctx m
            # needs v m-tile m -> vq blocks before those ctx MMs).
            vq_tiles = [None, None]

            def emit_vq(half, lo):
                def fn():
                    if lo == 0:
                        t = psS.tile([128, 2, 512], f32, tag="ps",
                                     name=f"vq{half}")
                        vq_tiles[half] = t.rearrange(
                            "p a (b x) -> p (a b) x", b=2)
                    v4 = vq_tiles[half]
                    v_quad_mm(v4, half, lo)
                    src = v4[:, lo:lo + 2, :].rearrange(
                        "p mi (h d) -> p mi h d", h=H_c)
                    nc.scalar.copy(
                        vh[:, half * 4 + lo:half * 4 + lo + 2, :, 64:128], src)
                return fn

            def emit_kps1():
                kps1 = psS.tile([128, 2, 512], f32, tag="ps", name="kps1")
                kq_pair_mm(kps1, wk_sb, xk_sb, 1)
                kq_copy(nc.scalar, kps1, kt_sb, 1)

            def emit_qps1():
                qps1 = psS.tile([128, 2, 512], f32, tag="ps", name="qps1")
                kq_pair_mm(qps1, wq_sb, xq_sb, 1)
                kq_copy(nc.scalar, qps1, qt_sb, 1)

            # pair-unit list, n0-major; half unit (parity 0 only, when
            # half_last) scheduled BEFORE the last full unit so the tail
            # after the final norm is a full-rate unit.
            units = [(hb, 0, False) for hb in range(HB)]
            if half_last:
                units += [(HB - 1, 1, True)] + \
                         [(hb, 1, False) for hb in range(HB - 1)]
            else:
                units += [(hb, 1, False) for hb in range(HB)]

            # slot[ui][m] -> emitters run right after exp/mult of m-tile m
            slots: dict = {
                0: {0: [emit_vq(0, 0)], 1: [emit_vq(0, 2)],
                    2: [emit_kps1], 3: [emit_vq(1, 0)], 5: [emit_vq(1, 2)]},
                1: {1: [emit_qps1]},
            }
            # C blocks for n-block 0 interleave into the n0=1 units; C blocks
            # for n-block 1 trail after the last unit.
            cq = list(range(4))
            for ui in (HB, HB + 1):
                if ui < len(units):
                    for mslot in (1, 4):
                        slots.setdefault(ui, {}).setdefault(mslot, []).append(
                            lambda nt=cq.pop(0): c_block(
                                nt, nc.scalar, [nc.sync]))

            for hb2, n02, half2 in units:
                if (hb2, n02) not in spd_tiles:
                    spd_issue(hb2, n02, half2)

            for ui, (hb, n0, half_unit) in enumerate(units):
                spds = spd_tiles.pop((hb, n0))

                ctxp = psC.tile([128, 2, 512], f32, tag="ctx")
                scs = {0: pair_scores(hb, n0, 0, half_unit),
                       1: pair_scores(hb, n0, 1, half_unit)}
                for m in range(MT):
                    e = pair_expmul(m, scs.pop(m), spds, nc.vector, half_unit)
                    for fn in slots.get(ui, {}).get(m, ()):
                        fn()
                    if m + 2 < MT:
                        # keep the PE score stream 2 m-tiles ahead of ACT
                        scs[m + 2] = pair_scores(hb, n0, m + 2, half_unit)
                    pair_ctx(hb, m, e, ctxp, half_unit,
                             start=(m == 0), stop=(m == MT - 1))
                pair_norm(hb, n0, ctxp, half_unit)

            # tail: out projection for the remaining n-blocks
            for nt in cq:
                c_block(nt, nc.scalar, [nc.sync])
            for nt in range(4, 8):
                c_block(nt, nc.scalar, [nc.sync, nc.scalar])

    nc.finalize()
    return nc


def _get_nc(H_c: int, n_kept: int, mode: str, half_last: bool):
    key = (H_c, n_kept, mode, half_last)
    if key not in _cache:
        _cache[key] = _build(H_c, HEADS / n_kept, mode, half_last)
    return _cache[key]


def _prep_inputs(x1, x2, spd, head_keep, Wq, Wkv, Wo, mode="bf16"):
    """Slice/transpose/pad host-side into per-core input maps."""
    import ml_dtypes

    ndt = np.float32 if mode in ("f32", "f32r") else ml_dtypes.bfloat16
    kept = [int(i) for i in np.nonzero(head_keep)[0]]
    n_kept = len(kept)
    half_last = (n_kept % 2 == 1)
    if not half_last:
        H_c = n_kept // 2
        if H_c % 2:
            H_c += 1
        groups = [kept[:H_c], kept[H_c:]]
    else:
        # odd count: both cores of a pair share the last kept head, each
        # computing one n-half of it (local column order differs per core)
        K = (n_kept - 1) // 2
        shared = kept[-1]
        H_c = K + 1
        pad = []
        if H_c % 2:
            H_c += 1
            pad = [None]
        groups = [kept[:K] + pad + [shared], kept[K:2 * K] + pad + [shared]]

    Wk_full, Wv_full = Wkv[:, :INNER], Wkv[:, INNER:]

    in_maps = []
    for b in range(B):
        xqT = np.ascontiguousarray(
            x1[b].T.reshape(KT, 128, N).transpose(1, 0, 2)).astype(ndt)
        xkT = np.ascontiguousarray(
            x2[b].T.reshape(KT, 128, N).transpose(1, 0, 2)).astype(ndt)
        for g in range(2):
            heads = groups[g]
            swap = half_last and g == 1  # local n0=0 <-> global half 1
            xq_g = xqT
            if swap:
                xq_g = np.ascontiguousarray(
                    np.concatenate([xqT[:, :, 512:], xqT[:, :, :512]], axis=2))
            HD = H_c * DIM_HEAD
            wq_c = np.zeros((DIM, HD), np.float32)
            wk_c = np.zeros((DIM, HD), np.float32)
            wv_c = np.zeros((DIM, HD), np.float32)
            wo_c = np.zeros((HD, DIM), np.float32)
            # exp(spd) multiplies exp(qk); absent slots get 1.0 (identity).
            # Interleaved per head-pair: [hb, n-block, part, m-tile, parity, n]
            spd_c = np.ones((H_c // 2, NB, 128, MT, 2, 512), ndt)
            for i, h in enumerate(heads):
                if h is None:
                    continue
                sl = slice(i * DIM_HEAD, (i + 1) * DIM_HEAD)
                hs = slice(h * DIM_HEAD, (h + 1) * DIM_HEAD)
                wq_c[:, sl] = Wq[:, hs] * SCALE
                wk_c[:, sl] = Wk_full[:, hs]
                wv_c[:, sl] = Wv_full[:, hs]
                wo_c[sl, :] = Wo[hs, :]
                # exp(spd[b,h]).T -> [n-block, partition, m-tile, n] image,
                # n-blocks in the core's LOCAL column order
                im = (np.exp(spd[b, h].T).reshape(MT, 128, NB, 512)
                      .transpose(2, 1, 0, 3)).astype(ndt)
                spd_c[i // 2, :, :, :, i % 2, :] = im[::-1] if swap else im
            in_maps.append({
                "xq": xq_g,
                "xk": xkT,
                "wq": np.ascontiguousarray(
                    wq_c.reshape(KT, 128, HD).transpose(1, 0, 2)).astype(ndt),
                "wk": np.ascontiguousarray(
                    wk_c.reshape(KT, 128, HD).transpose(1, 0, 2)).astype(ndt),
                "wv": np.ascontiguousarray(
                    wv_c.reshape(KT, 128, HD).transpose(1, 0, 2)).astype(ndt),
                "wo": np.ascontiguousarray(
                    wo_c.reshape(HD // 128, 128, DIM).transpose(1, 0, 2)).astype(ndt),
                "spd": spd_c,
            })
    return in_maps, n_kept, H_c


def _run(nc, in_maps, trace=False, tmpdir=None):
    from concourse.bass_utils import run_bass_kernel_spmd

    return run_bass_kernel_spmd(
        nc, in_maps, core_ids=list(range(NCORES)), trace=trace, tmpdir=tmpdir
    )


def kernel(x1, x2, spd, head_keep, Wq, Wkv, Wo, bo, _trace=False, _tmpdir=None):
    x1 = np.asarray(x1, np.float32)
    x2 = np.asarray(x2, np.float32)
    spd = np.asarray(spd, np.float32)
    head_keep = np.asarray(head_keep)
    n_kept = int(head_keep.astype(np.int64).sum())
    if n_kept == 0:
        # reference: 16/0 = inf, 0*inf = nan everywhere
        return np.full((B, N, DIM), np.nan, np.float32)

    mode = os.environ.get("KERNEL_DTYPE", "bf16")
    in_maps, n_kept, H_c = _prep_inputs(
        x1, x2, spd, head_keep, Wq, Wkv, Wo, mode)
    half_last = (n_kept % 2 == 1)
    nc = _get_nc(H_c, n_kept, mode, half_last)
    res = _run(nc, in_maps, trace=_trace, tmpdir=_tmpdir)

    out = np.empty((B, N, DIM), np.float32)
    bo32 = np.asarray(bo, np.float32)
    for b in range(B):
        o0 = res.results[2 * b]["out"].astype(np.float32)
        o1 = res.results[2 * b + 1]["out"].astype(np.float32)
        if half_last:
            o1 = np.concatenate([o1[512:], o1[:512]], axis=0)
        out[b] = o0 + o1 + bo32
    kernel._last_results = res
    return out


# revision 21
# speedup vs baseline: 1.1527x; 1.0792x over previous
"""Trainium2 Bass kernel for cross-attention with per-head structured mask.

Reference computation (B=4, N=1024, DIM=1024, H=16, D=64):
    q = x1 @ Wq;  k, v = split(x2 @ Wkv)
    dots = q k^T * D^-0.5 + spd
    attn = softmax(dots) * (head_keep * H / n_kept)   # whole heads dropped
    out  = (attn @ v) @ Wo + bo

Sharding: dropped heads contribute exactly zero, so only kept heads are
computed. Work unit = (batch b, kept-head group g): 8 cores = 4 batches x 2
head groups. Each core computes a partial out[b] (its heads' contribution
through Wo); host sums the two partials per batch and adds the bias.

Design (v3.2, ~93.5us vs 109us for the v1 phase-serial kernel measured
back-to-back on the same device):
  - Phase B runs over head-PAIR units (hb, n-block): both parities of one
    hb per m-tile.  The two K=64 score MMs sit at partition bases 0/64 and
    are emitted adjacently, so the PE executes them CONCURRENTLY in one
    ~259ns slot (row-tile auto-packing from base_partition).  One
    1024-wide exp (2-bank PSUM read, 1114ns vs 2x687) and one 1024-wide
    DVE espd multiply cover both heads; exp(s+p) = exp(s)*exp(p) keeps the
    spd bias out of the PE.  spd is shipped as exp(spd) bf16, interleaved
    per head-pair ([hb, n0, part, m, parity, n]).
  - Phase A is software-pipelined INTO phase B: only K/Q projections of
    the first halves precede the exp stream; V projection and the
    remaining projection groups drop into slot positions between B steps
    (PE queue filler).  Out-projection for n-block 0 interleaves into the
    n-block-1 units; only n-block 1's out-proj trails the last exp.
  - V-augmentation: ones columns always 0:63, values 64:127, so the
    softmax denominator lands at PSUM partition base 0 for every head
    (reciprocal_approx_fast is base-0-only) and V copy-out is one strided
    op per chain-quad.  Two V chains share a PSUM bank: only the bank's
    first MM carries start=True (start clears the WHOLE bank's
    has_written bits; the second chain lands via has_written=0 overwrite).
  - Engine division: PE matmuls; ACT (scalar) exps + all PSUM->SBUF
    evacuations; DVE espd multiplies + softmax normalization; sync+gpsimd
    issue DMA (scalar queue issues none after v3.1 -- its queue is
    latency-critical).  GpSimd tensor_tensor was tried for the multiply
    and is a net loss (2.2us/op + ~500ns sem latency poisons the ctx
    dependency chain).
  - PSUM: psS pool [128,2,512] bufs=3 (score pairs + projection chains +
    out-proj pairs, 6 banks) + psC bufs=1 (paired ctx chains, 2 banks).
  - Startup DMA: 128KB-granular descriptors round-robin on sync/gpsimd in
    dependency-priority order; per-descriptor throughput is only
    ~15-55GB/s so aggregate bandwidth needs 15+ descriptors in flight.
    spd tiles prefetch 2 units ahead (deeper prefetch steals ramp
    bandwidth from the critical K/Q/V loads and regresses: measured
    99.9us with all-tiles-upfront vs 93.6us).

HW facts (measured this session, micro-benched):
  - warm PE pace: N=512 bf16 MM ~216-259ns in a dense stream (fill/drain
    overlap); isolated MM ~379ns; two K=64 MMs at bases 0/64 share one
    slot (verified 3-4ns apart).  LDWEIGHTS alternation is free.
  - ACTIVATE ~= (N+352)/1.2 ns: 512->687, 1024->1114, 2048->1967.
    Multi-bank PSUM reads by ACT/DVE are legal and correct.
  - DVE tensor_tensor bf16 1024-wide ~620-1070ns (2x_1p mode).
  - DMA accum_op (CCE) does NOT support mult (compiler verifier rejects).
  - engines execute queues IN PROGRAM ORDER; emission order is the
    schedule.  Cross-engine dependency latency (sem prop + queue) is
    ~200-500ns; structure so the PE always has ready work queued between
    dependent ops.
  - fp8 espd fails the 2e-2 absmax gate (2.16e-2 even half-fp8); bf16 is
    the floor (rel err 8.2e-3).
  - sustained PE clock drops to ~2.0GHz (P0); device drifts ~10-20%
    slower when hot from repeated benching.
"""

import os

import numpy as np

B, N, DIM = 4, 1024, 1024
HEADS, DIM_HEAD = 16, 64
INNER = HEADS * DIM_HEAD
SCALE = DIM_HEAD ** -0.5
NCORES = 8
KT = DIM // 128      # 8 contraction tiles
NB = N // 512        # 2 column blocks
MT = N // 128        # 8 key tiles

_cache: dict = {}


def _build(H_c: int, keep_scale: float, mode: str = "bf16", half_last: bool = False):
    """Build + compile the per-core Bass program for H_c heads (H_c even)."""
    import concourse.mybir as mybir
    import concourse.tile as tile
    from concourse import bacc

    dt = mybir.dt
    f32 = dt.float32
    HB = H_c // 2
    HD = H_c * DIM_HEAD
    # the v3 pair-unit schedule hardcodes slot positions for HB == 2
    assert H_c == 4, f"v3 schedule requires H_c == 4, got {H_c}"

    mmdt = {"bf16": dt.bfloat16, "f32r": dt.float32r, "f32": f32}[mode]

    nc = bacc.Bacc("TRN2", target_bir_lowering=False)

    xq = nc.dram_tensor("xq", [128, KT, N], mmdt, kind="ExternalInput")   # x1[b].T, sbuf image
    xk = nc.dram_tensor("xk", [128, KT, N], mmdt, kind="ExternalInput")   # x2[b].T, sbuf image
    wq = nc.dram_tensor("wq", [128, KT, HD], mmdt, kind="ExternalInput")  # sbuf layout
    wk = nc.dram_tensor("wk", [128, KT, HD], mmdt, kind="ExternalInput")
    wv = nc.dram_tensor("wv", [128, KT, HD], mmdt, kind="ExternalInput")
    wo = nc.dram_tensor("wo", [128, HD // 128, DIM], mmdt, kind="ExternalInput")
    # exp(spd[b,h]).T images, interleaved per head-PAIR: index [hb, n-block,
    # partition, m-tile, parity, n] so one [128, 2, 512] slice covers both
    # heads of a pair for one m-tile (feeds one 1024-wide exp/mult batch)
    spddt = mmdt
    spd = nc.dram_tensor("spd", [HB, NB, 128, MT, 2, 512], spddt,
                         kind="ExternalInput")
    # bf16 partials: host sums the two per-batch partials in fp32
    out = nc.dram_tensor("out", [N, DIM], dt.bfloat16, kind="ExternalOutput")

    Exp = mybir.ActivationFunctionType.Exp
    mult = mybir.AluOpType.mult

    # DMA issue queues, round-robined for startup concurrency
    with tile.TileContext(nc) as tc:
        with (
            tc.tile_pool(name="w", bufs=1) as wpool,
            tc.tile_pool(name="big", bufs=1) as big,
            tc.tile_pool(name="spdp", bufs=3) as spdp,
            tc.tile_pool(name="es", bufs=3) as espool,
            tc.tile_pool(name="ep", bufs=4) as epool,
            tc.tile_pool(name="nrm", bufs=2) as nrm,
            tc.tile_pool(name="op", bufs=3) as opool,
            tc.tile_pool(name="psS", bufs=3, space="PSUM") as psS,
            tc.tile_pool(name="psC", bufs=1, space="PSUM") as psC,
        ):
            wq_sb = wpool.tile([128, KT, HD], mmdt, tag="wq")
            wk_sb = wpool.tile([128, KT, HD], mmdt, tag="wk")
            wv_sb = wpool.tile([128, KT, HD], mmdt, tag="wv")
            wo_sb = wpool.tile([128, HD // 128, DIM], mmdt, tag="wo")
            xk_sb = big.tile([128, KT, N], mmdt, tag="xkim")
            xq_sb = big.tile([128, KT, N], mmdt, tag="xqim")

            qt_sb = big.tile([128, HB, N], mmdt, tag="qt")
            kt_sb = big.tile([128, HB, N], mmdt, tag="kt")
            v_sb = big.tile([128, MT, H_c * 128], mmdt, tag="v")
            ct_sb = big.tile([128, HB, N], mmdt, tag="ct")
            vh = v_sb.rearrange("p m (h x) -> p m h x", h=H_c)

            # ---- startup DMA: dependency-priority order, 128KB slices,
            # round-robin over the three issue-capable queues ----
            qs = [nc.sync, nc.gpsimd]
            qi = [0]

            def issue(dst, src):
                qs[qi[0] % 2].dma_start(dst, src)
                qi[0] += 1

            issue(wk_sb[:, 0:KT // 2, :], wk[:, 0:KT // 2, :])
            issue(wk_sb[:, KT // 2:, :], wk[:, KT // 2:, :])
            for k in range(KT):
                issue(xk_sb[:, k, 0:512], xk[:, k, 0:512])
            issue(wq_sb[:, 0:KT // 2, :], wq[:, 0:KT // 2, :])
            issue(wq_sb[:, KT // 2:, :], wq[:, KT // 2:, :])
            issue(wv_sb[:, 0:KT // 2, :], wv[:, 0:KT // 2, :])
            issue(wv_sb[:, KT // 2:, :], wv[:, KT // 2:, :])
            for k in range(KT):
                issue(xq_sb[:, k, 0:512], xq[:, k, 0:512])
            # spd pair-unit tiles: [128, MT, 2, 512] (full pair) or
            # [128, MT, 512] strided fetch of parity 0 only (half unit)
            spd_tiles: dict = {}

            def spd_issue(hb, n0, half_unit):
                if half_unit:
                    t = spdp.tile([128, MT, 512], spddt, tag="spd",
                                  name=f"spd{hb}_{n0}h")
                    for qq in range(4):
                        nc.sync.dma_start(
                            t[:, 2 * qq:2 * qq + 2, :],
                            spd[hb, n0, :, 2 * qq:2 * qq + 2, 0, :])
                else:
                    t = spdp.tile([128, MT, 2, 512], spddt, tag="spd",
                                  name=f"spd{hb}_{n0}")
                    for qq in range(4):
                        nc.sync.dma_start(
                            t[:, 2 * qq:2 * qq + 2, :, :],
                            spd[hb, n0, :, 2 * qq:2 * qq + 2, :, :])
                spd_tiles[(hb, n0)] = t
                return t

            spd_issue(0, 0, False)
            for k in range(KT):
                issue(xk_sb[:, k, 512:N], xk[:, k, 512:N])
            if HB > 1:
                spd_issue(1, 0, False)
            issue(wo_sb[:, :, 0:512], wo[:, :, 0:512])
            issue(wo_sb[:, :, 512:1024], wo[:, :, 512:1024])
            for k in range(KT):
                issue(xq_sb[:, k, 512:N], xq[:, k, 512:N])

            # ones columns of the augmented V blocks: always cols 0:63 so
            # the sumexp rows always land at PSUM partition base 0
            nc.gpsimd.memset(vh[:, :, :, 0:64], 1.0)
            if half_last:
                # the shared head's slot is only computed at local n0=0; its
                # n0=1 region of ctxT must read as zero in the out projection
                nc.gpsimd.memset(ct_sb[64:128, HB - 1, 512:N], 0.0)

            # ---- phase A building blocks (each: matmul group + evacuation) ----
            def kq_pair_mm(ps, w_sb, x_im, half):
                """16 MMs: project K or Q for both hb blocks of one 512-col
                half.  ps: [128,2,512] psum tile."""
                sl = slice(half * 512, (half + 1) * 512)
                for k in range(KT):
                    for hb in range(HB):
                        nc.tensor.matmul(
                            ps[:, hb, :],
                            w_sb[:, k, hb * 128:(hb + 1) * 128],
                            x_im[:, k, sl],
                            start=(k == 0), stop=(k == KT - 1),
                        )

            def kq_copy(eng, ps, dst_sb, half):
                sl = slice(half * 512, (half + 1) * 512)
                if eng is nc.vector:
                    eng.tensor_copy(dst_sb[:, 0:HB, sl], ps[:, 0:HB, :])
                else:
                    eng.copy(dst_sb[:, 0:HB, sl], ps[:, 0:HB, :])

            def v_quad_mm(ps4, half, lo):
                """16 MMs: V projection for m-tiles half*4+lo .. +lo+1.
                ps4: [128,4,256] view of a [128,2,512] psum tile.  The two
                chains share one PSUM bank and start=True clears the WHOLE
                bank's has_written bits, so only the very first MM into the
                bank may carry start; the second chain's k=0 write lands via
                has_written=0 overwrite semantics."""
                sl = slice(half * 512, (half + 1) * 512)
                for k in range(KT):
                    for mi in (lo, lo + 1):
                        nc.tensor.matmul(
                            ps4[:, mi, :],
                            xk_sb[:, k, sl][:, mi * 128:(mi + 1) * 128],
                            wv_sb[:, k, :],
                            start=(k == 0 and mi == lo),
                            stop=(k == KT - 1),
                        )

            def v_quad_copy(eng, ps4, half):
                src = ps4.rearrange("p mi (h d) -> p mi h d", h=H_c)
                dst = vh[:, half * 4:(half + 1) * 4, :, 64:128]
                if eng is nc.vector:
                    eng.tensor_copy(dst, src)
                else:
                    eng.copy(dst, src)

            # ---- phase B: head-PAIR units.  Both parities of one hb are
            # processed together per m-tile: the two K=64 score MMs sit at
            # partition bases 0 and 64, so the PE runs them CONCURRENTLY in
            # one ~259ns slot (row-tile auto-packing); one 1024-wide exp and
            # one 1024-wide espd multiply cover both heads. ----
            def pair_scores(hb, n0, m, half_unit):
                n_sl = slice(n0 * 512, (n0 + 1) * 512)
                sc = psS.tile([128, 2, 512], f32, tag="ps",
                              name=f"sc{hb}_{n0}_{m}")
                nc.tensor.matmul(
                    sc[:, 0, :],
                    kt_sb[0:64, hb, m * 128:(m + 1) * 128],
                    qt_sb[0:64, hb, n_sl],
                    start=True, stop=True,
                )
                if not half_unit:
                    nc.tensor.matmul(
                        sc[:, 1, :],
                        kt_sb[64:128, hb, m * 128:(m + 1) * 128],
                        qt_sb[64:128, hb, n_sl],
                        start=True, stop=True,
                    )
                return sc

            def pair_expmul(m, sc, spds, mul_eng, half_unit):
                if half_unit:
                    es = espool.tile([128, 2, 512], mmdt, tag="es")
                    nc.scalar.activation(es[:, 0, :], sc[:, 0, :], Exp)
                    e = epool.tile([128, 2, 512], mmdt, tag="e")
                    mul_eng.tensor_tensor(
                        e[:, 0, :], es[:, 0, :], spds[:, m, :], op=mult)
                    return e
                es = espool.tile([128, 2, 512], mmdt, tag="es")
                nc.scalar.activation(
                    es[:].rearrange("p a b -> p (a b)"),
                    sc[:].rearrange("p a b -> p (a b)"), Exp)
                e = epool.tile([128, 2, 512], mmdt, tag="e")
                mul_eng.tensor_tensor(
                    e[:].rearrange("p a b -> p (a b)"),
                    es[:].rearrange("p a b -> p (a b)"),
                    spds[:, m, :, :].rearrange("p a b -> p (a b)"),
                    op=mult)
                return e

            def pair_ctx(hb, m, e, ctxp, half_unit, start, stop):
                for par in range(1 if half_unit else 2):
                    h = 2 * hb + par
                    nc.tensor.matmul(
                        ctxp[:, par, :],
                        v_sb[:, m, h * 128:(h + 1) * 128],
                        e[:, par, :],
                        start=start,
                        stop=stop,
                    )

            def pair_norm(hb, n0, ctxp, half_unit):
                n_sl = slice(n0 * 512, (n0 + 1) * 512)
                rr = nrm.tile([128, 2, 512], f32, tag="rr")
                if half_unit:
                    nc.vector.reciprocal_approx_fast(
                        rr[0:64, 0, :], ctxp[0:64, 0, :])
                else:
                    nc.vector.reciprocal_approx_fast(
                        rr[0:64, :, :].rearrange("p a b -> p (a b)"),
                        ctxp[0:64, :, :].rearrange("p a b -> p (a b)"))
                for par in range(1 if half_unit else 2):
                    nc.vector.scalar_tensor_tensor(
                        out=ct_sb[par * 64:par * 64 + 64, hb, n_sl],
                        in0=ctxp[64:128, par, :],
                        scalar=float(keep_scale),
                        in1=rr[0:64, par, :],
                        op0=mult,
                        op1=mult,
                    )

            # ---- phase C: out projection for one row-tile ----
            def c_block(nt, cast_eng, wr_eng):
                po = psS.tile([128, 2, 512], f32, tag="ps", name=f"po{nt}")
                for d0 in range(2):
                    for kk in range(HD // 128):
                        nc.tensor.matmul(
                            po[:, d0, :],
                            ct_sb[:, kk, nt * 128:(nt + 1) * 128],
                            wo_sb[:, kk, d0 * 512:(d0 + 1) * 512],
                            start=(kk == 0),
                            stop=(kk == HD // 128 - 1),
                        )
                ot = opool.tile([128, 2, 512], dt.bfloat16, tag="o")
                if cast_eng is nc.vector:
                    cast_eng.tensor_copy(
                        ot[:].rearrange("p a b -> p (a b)"),
                        po[:].rearrange("p a b -> p (a b)"))
                else:
                    cast_eng.copy(
                        ot[:].rearrange("p a b -> p (a b)"),
                        po[:].rearrange("p a b -> p (a b)"))
                for d0 in range(2):
                    if len(wr_eng) > 1:
                        for rh in range(2):
                            wr_eng[(2 * d0 + rh) % len(wr_eng)].dma_start(
                                out[nt * 128 + rh * 64:nt * 128 + rh * 64 + 64,
                                    d0 * 512:(d0 + 1) * 512],
                                ot[rh * 64:rh * 64 + 64, d0, :])
                    else:
                        wr_eng[0].dma_start(
                            out[nt * 128:(nt + 1) * 128,
                                d0 * 512:(d0 + 1) * 512],
                            ot[:, d0, :])

            # =========== EMISSION SCHEDULE ===========
            # Phase A prefix: ONLY K half 0 + Q half 0 precede phase B (the
            # V projection and all remaining A groups drop into slot
            # positions inside the B units, keeping the PE queue dense).
            # PE is the binding engine (~259ns/N=512 slot, K=64 score pairs
            # share a slot), so ACT/DVE idling mid-stream is acceptable;
            # the PE queue must never wait.
            kps0 = psS.tile([128, 2, 512], f32, tag="ps", name="kps0")
            kq_pair_mm(kps0, wk_sb, xk_sb, 0)
            kq_copy(nc.scalar, kps0, kt_sb, 0)

            qps0 = psS.tile([128, 2, 512], f32, tag="ps", name="qps0")
            kq_pair_mm(qps0, wq_sb, xq_sb, 0)
            kq_copy(nc.scalar, qps0, qt_sb, 0)

            # Remaining A groups as slot emitters (dependency notes: scores
            # for m>=4 need kt half 1 -> kps1 EMITTED before them; ctx m
            # needs v m-tile m -> vq blocks before those ctx MMs).
            vq_tiles = [None, None]

            def emit_vq(half, lo):
                def fn():
                    if lo == 0:
                        t = psS.tile([128, 2, 512], f32, tag="ps",
                                     name=f"vq{half}")
                        vq_tiles[half] = t.rearrange(
                            "p a (b x) -> p (a b) x", b=2)
                    v4 = vq_tiles[half]
                    v_quad_mm(v4, half, lo)
                    src = v4[:, lo:lo + 2, :].rearrange(
                        "p mi (h d) -> p mi h d", h=H_c)
                    nc.scalar.copy(
                        vh[:, half * 4 + lo:half * 4 + lo + 2, :, 64:128], src)
                return fn

            def emit_kps1():
                kps1 = psS.tile([128, 2, 512], f32, tag="ps", name="kps1")
                kq_pair_mm(kps1, wk_sb, xk_sb, 1)
                kq_copy(nc.scalar, kps1, kt_sb, 1)

            def emit_qps1():
                qps1 = psS.tile([128, 2, 512], f32, tag="ps", name="qps1")
                kq_pair_mm(qps1, wq_sb, xq_sb, 1)
                kq_copy(nc.scalar, qps1, qt_sb, 1)

            # pair-unit list, n0-major; half unit (parity 0 only, when
            # half_last) scheduled BEFORE the last full unit so the tail
            # after the final norm is a full-rate unit.
            units = [(hb, 0, False) for hb in range(HB)]
            if half_last:
                units += [(HB - 1, 1, True)] + \
                         [(hb, 1, False) for hb in range(HB - 1)]
            else:
                units += [(hb, 1, False) for hb in range(HB)]

            # slot[ui][m] -> emitters run right after exp/mult of m-tile m
            slots: dict = {
                0: {0: [emit_vq(0, 0)], 1: [emit_vq(0, 2)],
                    2: [emit_kps1], 3: [emit_vq(1, 0)], 5: [emit_vq(1, 2)]},
                1: {1: [emit_qps1]},
            }
            # C blocks for n-block 0 interleave into the n0=1 units; C blocks
            # for n-block 1 trail after the last unit.
            cq = list(range(4))
            for ui in (HB, HB + 1):
                if ui < len(units):
                    for mslot in (1, 4):
                        slots.setdefault(ui, {}).setdefault(mslot, []).append(
                            lambda nt=cq.pop(0): c_block(
                                nt, nc.scalar, [nc.sync]))

            for ui, (hb, n0, half_unit) in enumerate(units):
                if (hb, n0) in spd_tiles:
                    spds = spd_tiles.pop((hb, n0))
                else:
                    spds = spd_issue(hb, n0, half_unit)
                # prefetch two units ahead
                ahead = ui + 2
                if ahead < len(units):
                    ah, an, ahalf = units[ahead]
                    if (ah, an) not in spd_tiles:
                        spd_issue(ah, an, ahalf)

                ctxp = psC.tile([128, 2, 512], f32, tag="ctx")
                scs = {0: pair_scores(hb, n0, 0, half_unit),
                       1: pair_scores(hb, n0, 1, half_unit)}
                for m in range(MT):
                    e = pair_expmul(m, scs.pop(m), spds, nc.vector, half_unit)
                    for fn in slots.get(ui, {}).get(m, ()):
                        fn()
                    if m + 2 < MT:
                        # keep the PE score stream 2 m-tiles ahead of ACT
                        scs[m + 2] = pair_scores(hb, n0, m + 2, half_unit)
                    pair_ctx(hb, m, e, ctxp, half_unit,
                             start=(m == 0), stop=(m == MT - 1))
                pair_norm(hb, n0, ctxp, half_unit)

            # tail: out projection for the remaining n-blocks
            for nt in cq:
                c_block(nt, nc.scalar, [nc.sync])
            for nt in range(4, 8):
                c_block(nt, nc.scalar, [nc.sync, nc.scalar])

    nc.finalize()
    return nc


def _get_nc(H_c: int, n_kept: int, mode: str, half_last: bool):
    key = (H_c, n_kept, mode, half_last)
    if key not in _cache:
        _cache[key] = _build(H_c, HEADS / n_kept, mode, half_last)
    return _cache[key]


def _prep_inputs(x1, x2, spd, head_keep, Wq, Wkv, Wo, mode="bf16"):
    """Slice/transpose/pad host-side into per-core input maps."""
    import ml_dtypes

    ndt = np.float32 if mode in ("f32", "f32r") else ml_dtypes.bfloat16
    kept = [int(i) for i in np.nonzero(head_keep)[0]]
    n_kept = len(kept)
    half_last = (n_kept % 2 == 1)
    if not half_last:
        H_c = n_kept // 2
        if H_c % 2:
            H_c += 1
        groups = [kept[:H_c], kept[H_c:]]
    else:
        # odd count: both cores of a pair share the last kept head, each
        # computing one n-half of it (local column order differs per core)
        K = (n_kept - 1) // 2
        shared = kept[-1]
        H_c = K + 1
        pad = []
        if H_c % 2:
            H_c += 1
            pad = [None]
        groups = [kept[:K] + pad + [shared], kept[K:2 * K] + pad + [shared]]

    Wk_full, Wv_full = Wkv[:, :INNER], Wkv[:, INNER:]

    in_maps = []
    for b in range(B):
        xqT = np.ascontiguousarray(
            x1[b].T.reshape(KT, 128, N).transpose(1, 0, 2)).astype(ndt)
        xkT = np.ascontiguousarray(
            x2[b].T.reshape(KT, 128, N).transpose(1, 0, 2)).astype(ndt)
        for g in range(2):
            heads = groups[g]
            swap = half_last and g == 1  # local n0=0 <-> global half 1
            xq_g = xqT
            if swap:
                xq_g = np.ascontiguousarray(
                    np.concatenate([xqT[:, :, 512:], xqT[:, :, :512]], axis=2))
            HD = H_c * DIM_HEAD
            wq_c = np.zeros((DIM, HD), np.float32)
            wk_c = np.zeros((DIM, HD), np.float32)
            wv_c = np.zeros((DIM, HD), np.float32)
            wo_c = np.zeros((HD, DIM), np.float32)
            # exp(spd) multiplies exp(qk); absent slots get 1.0 (identity).
            # Interleaved per head-pair: [hb, n-block, part, m-tile, parity, n]
            spd_c = np.ones((H_c // 2, NB, 128, MT, 2, 512), ndt)
            for i, h in enumerate(heads):
                if h is None:
                    continue
                sl = slice(i * DIM_HEAD, (i + 1) * DIM_HEAD)
                hs = slice(h * DIM_HEAD, (h + 1) * DIM_HEAD)
                wq_c[:, sl] = Wq[:, hs] * SCALE
                wk_c[:, sl] = Wk_full[:, hs]
                wv_c[:, sl] = Wv_full[:, hs]
                wo_c[sl, :] = Wo[hs, :]
                # exp(spd[b,h]).T -> [n-block, partition, m-tile, n] image,
                # n-blocks in the core's LOCAL column order
                im = (np.exp(spd[b, h].T).reshape(MT, 128, NB, 512)
                      .transpose(2, 1, 0, 3)).astype(ndt)
                spd_c[i // 2, :, :, :, i % 2, :] = im[::-1] if swap else im
            in_maps.append({
                "xq": xq_g,
                "xk": xkT,
                "wq": np.ascontiguousarray(
                    wq_c.reshape(KT, 128, HD).transpose(1, 0, 2)).astype(ndt),
                "wk": np.ascontiguousarray(
                    wk_c.reshape(KT, 128, HD).transpose(1, 0, 2)).astype(ndt),
                "wv": np.ascontiguousarray(
                    wv_c.reshape(KT, 128, HD).transpose(1, 0, 2)).astype(ndt),
                "wo": np.ascontiguousarray(
                    wo_c.reshape(HD // 128, 128, DIM).transpose(1, 0, 2)).astype(ndt),
                "spd": spd_c,
            })
    return in_maps, n_kept, H_c


def _run(nc, in_maps, trace=False, tmpdir=None):
    from concourse.bass_utils import run_bass_kernel_spmd

    return run_bass_kernel_spmd(
        nc, in_maps, core_ids=list(range(NCORES)), trace=trace, tmpdir=tmpdir
    )


def kernel(x1, x2, spd, head_keep, Wq, Wkv, Wo, bo, _trace=False, _tmpdir=None):
    x1 = np.asarray(x1, np.float32)
    x2 = np.asarray(x2, np.float32)
    spd = np.asarray(spd, np.float32)
    head_keep = np.asarray(head_keep)
    n_kept = int(head_keep.astype(np.int64).sum())
    if n_kept == 0:
        # reference: 16/0 = inf, 0*inf = nan everywhere
        return np.full((B, N, DIM), np.nan, np.float32)

    mode = os.environ.get("KERNEL_DTYPE", "bf16")
    in_maps, n_kept, H_c = _prep_inputs(
        x1, x2, spd, head_keep, Wq, Wkv, Wo, mode)
    half_last = (n_kept % 2 == 1)
    nc = _get_nc(H_c, n_kept, mode, half_last)
    res = _run(nc, in_maps, trace=_trace, tmpdir=_tmpdir)

    out = np.empty((B, N, DIM), np.float32)
    bo32 = np.asarray(bo, np.float32)
    for b in range(B):
        o0 = res.results[2 * b]["out"].astype(np.float32)
        o1 = res.results[2 * b + 1]["out"].astype(np.float32)
        if half_last:
            o1 = np.concatenate([o1[512:], o1[:512]], axis=0)
        out[b] = o0 + o1 + bo32
    kernel._last_results = res
    return out
